# revision 1
# baseline (speedup 1.0000x reference)
"""Trainium2 Bass kernel for nn_Attention_50989851738305.

The reference module applies jnp.tril(scores, k=-999999) which zeroes the
entire score matrix (S=2048 << 999999), so softmax is uniform 1/S and the
attention output reduces exactly to

    out[b, s, :] = (mean_s' hidden[b, s', :]) @ Wv.T @ Wo.T   (constant in s)

Wq/Wk are mathematically irrelevant. The kernel distributes over 8 cores:
  - sequence dim sharded 8x for reading hidden + writing output,
  - inner (head) dim sharded 8x for the Wv/Wo weight work,
  - two 8KB AllReduces stitch the partial sums / partial outputs together.
"""
import numpy as np

import concourse.bass as bass  # noqa: F401  (bass registers engine types)
import concourse.tile as tile
from concourse import bacc, mybir
from concourse.bass_utils import run_bass_kernel_spmd

B = 2
S = 2048
D = 1024
N_CORES = 8
S_LOC = S // N_CORES      # 256 sequence rows per core
J_LOC = D // N_CORES      # 128 inner (head) columns per core
SCALE = 1.0 / S           # uniform attention weight (exact power of two)
F32 = mybir.dt.float32

_BUILT = {}
NO_COLLECTIVE = False  # timing experiment: replace AllReduce with local copy
DMA_ONLY = False       # timing experiment: loads + stores only, no compute
NO_AR1 = False         # timing experiment: skip AllReduce #1 only
NO_AR2 = False         # timing experiment: skip AllReduce #2 only
REMOTE_AR1 = False     # replace AllReduce #1 with remote_dma XOR all-gather
_ITER = [0]            # front-emission counter for cumulative sem targets


def _emit_body(nc, tc, pools, const, h_d, out_d):
    pre = _emit_front(nc, tc, pools, const, h_d, out_d)
    if pre is not None:
        _emit_back(nc, tc, pools, pre, out_d)


def _emit_const(nc, tc, pool, wvt_d, wot_d):
    """Once-per-kernel: constants + resident weight slices."""
    DC = D // 128
    ones_col = pool.tile([128, 1], F32, tag="ones_col", bufs=1)  # value = 1/S
    nc.vector.memset(ones_col[:], SCALE)
    # weight slices (pre-transposed on host): wvt [D, J_LOC], wot [J_LOC, D]
    wvt_sb = pool.tile([128, DC, J_LOC], F32, tag="wvt", bufs=1)
    nc.sync.dma_start(wvt_sb[:], wvt_d.ap().rearrange("(c p) j -> p c j", p=128))
    wot_sb = pool.tile([J_LOC, D], F32, tag="wot", bufs=1)
    nc.sync.dma_start(wot_sb[:], wot_d[:])
    return ones_col, wvt_sb, wot_sb


def _emit_front(nc, tc, pools, const, h_d, out_d):
    """Loads + local partial sums + AllReduce #1. Returns handles for the
    back half (or None in DMA_ONLY mode)."""
    pool, psum1, psum2, dram = pools
    ones_col, wvt_sb, wot_sb = const
    NCH = B * S_LOC // 128    # 4 sbuf row-chunks of hidden
    DC = D // 128             # 8 chunks of the model dim
    group = [list(range(N_CORES))]

    # hidden slice [B*S_LOC, D] in one DMA: [128, 4, D], col-chunk c = rows
    h_big = pool.tile([128, NCH, D], F32, tag="hbig")
    nc.sync.dma_start(h_big[:], h_d.ap().rearrange("(c p) d -> p c d", p=128))
    h_sb = [h_big[:, c, :] for c in range(NCH)]

    if DMA_ONLY:
        nc.scalar.dma_start(out_d[0 * 128:1 * 128, :], h_sb[0][:])
        nc.scalar.dma_start(out_d[1 * 128:2 * 128, :], h_sb[1][:])
        nc.scalar.dma_start(out_d[2 * 128:3 * 128, :], h_sb[2][:])
        nc.scalar.dma_start(out_d[3 * 128:4 * 128, :], h_sb[3][:])
        unused = pool.tile([128, 2], F32, tag="unused")
        nc.vector.tensor_copy(unused[:, 0:1], wvt_sb[:, 0, 0:1])
        nc.vector.tensor_copy(unused[:, 1:2], wot_sb[:, 0:1])
        cc0 = dram.tile([128, 2], F32, tag="cc0")
        nc.sync.dma_start(cc0[:], unused[:])
        return None

    # fold the two 128-row chunks of each batch on DVE first (halves PE work)
    hsum = []
    for b in range(B):
        t = pool.tile([128, D], F32, tag=f"hs{b}")
        nc.vector.tensor_tensor(t[:], h_sb[2 * b][:], h_sb[2 * b + 1][:],
                                mybir.AluOpType.add)
        hsum.append(t)

    # partial column sums of hidden, transposed layout:
    # pT[p, dc*2 + b] = (1/S) * sum_{s in local slice} h[b, s, dc*128 + p]
    pT_psum = psum1.tile([128, 2 * DC], F32, tag="pT")
    for b in range(B):
        for dc in range(DC):
            col = dc * 2 + b
            nc.tensor.matmul(
                pT_psum[:, col:col + 1],
                hsum[b][:, dc * 128:(dc + 1) * 128],
                ones_col[:],
                start=True,
                stop=True,
            )

    if REMOTE_AR1:
        it = _ITER[0]
        _ITER[0] += 1
        # XOR-slot all-gather: slot k of the gather tile receives from the
        # XOR-k peer, so the destination AP is compile-time constant and
        # collision-free. Remaining ncfw AllReduce #2 is the rendezvous that
        # bounds inter-core skew < 1 iteration, making double-buffered slot
        # reuse safe without an ack round.
        g1 = pool.tile([128, 8, 2 * DC], F32, tag="g1")
        nc.vector.tensor_copy(g1[:, 0, :], pT_psum[:])    # self slot
        with tc.tile_critical():
            for k in range(1, 8):
                rdests = [None] * 8
                rdests[k] = (0, k)
                nc.gpsimd.remote_dma_broadcast(
                    g1[:, k, :], g1[:, 0, :],
                    remote_sem=nc._rs1, local_sem=nc._ls1,
                    rdests=rdests,
                ).then_inc(nc._prep1, 1)
            nc.gpsimd.wait_ge(nc._prep1, 7 * (it + 1))
            nc.gpsimd.trigger_dma(7)
            nc.gpsimd.wait_ge(nc._ls1, 112 * (it + 1))
        pT_sb = pool.tile([128, 2 * DC], F32, tag="pTs")
        with tc.tile_critical():
            nc.vector.wait_ge(nc._rs1, 14 * (it + 1))
            nc.vector.tensor_reduce(
                pT_sb[:], g1[:].rearrange("p k f -> p f k"),
                mybir.AxisListType.X, mybir.AluOpType.add,
            )
        return pT_sb, wvt_sb, wot_sb

    # AllReduce #1: full-sequence mean (transposed layout), 8KB
    cc1_in = dram.tile([128, 2 * DC], F32, tag="cc1i")
    cc1_out = dram.tile([128, 2 * DC], F32, tag="cc1o", addr_space="Shared")
    pT_loc = pool.tile([128, 2 * DC], F32, tag="pTl")
    nc.vector.tensor_copy(pT_loc[:], pT_psum[:])
    nc.scalar.dma_start(cc1_in[:], pT_loc[:])
    if NO_COLLECTIVE or NO_AR1:
        nc.gpsimd.dma_start(cc1_out[:], cc1_in[:])
    else:
        nc.gpsimd.collective_compute(
            "AllReduce", mybir.AluOpType.add, replica_groups=group,
            ins=[cc1_in.opt()], outs=[cc1_out.opt()],
        )
    pT_sb = pool.tile([128, 2 * DC], F32, tag="pTs")
    nc.scalar.dma_start(pT_sb[:], cc1_out[:])
    return pT_sb, wvt_sb, wot_sb


def _emit_back(nc, tc, pools, pre, out_d):
    """Everything after AllReduce #1: weight matmuls, AllReduce #2, output."""
    pool, psum1, psum2, dram = pools
    pT_sb, wvt_sb, wot_sb = pre
    DC = D // 128
    group = [list(range(N_CORES))]

    # yT[j, b] = sum_d wvt[d, j] * mT[d, b]   (local j slice of 128)
    yT_psum = psum1.tile([128, B], F32, tag="yT")
    for dc in range(DC):
        nc.tensor.matmul(
            yT_psum[:],
            wvt_sb[:, dc, :],
            pT_sb[:, dc * 2:dc * 2 + 2],
            start=(dc == 0),
            stop=(dc == DC - 1),
        )
    yT_sb = pool.tile([128, B], F32, tag="yTs")
    nc.vector.tensor_copy(yT_sb[:], yT_psum[:])

    # r[b, :] partial = y[b, jslice] @ wot[jslice, :]  (natural layout)
    r_psum = [psum2.tile([1, D], F32, tag="rwork", name=f"rn{b}")
              for b in range(B)]
    for b in range(B):
        for nf in range(2):
            nc.tensor.matmul(
                r_psum[b][0:1, nf * 512:(nf + 1) * 512],
                yT_sb[:, b:b + 1],
                wot_sb[:, nf * 512:(nf + 1) * 512],
                start=True,
                stop=True,
            )

    # AllReduce #2: combine partial output rows over the j shards, 8KB
    cc2_in = dram.tile([B, D], F32, tag="cc2i")
    cc2_out = dram.tile([B, D], F32, tag="cc2o", addr_space="Shared")
    r_loc = [pool.tile([1, D], F32, tag=f"rl{b}", name=f"rl{b}")
             for b in range(B)]
    nc.vector.tensor_copy(r_loc[0][:], r_psum[0][:])
    nc.scalar.copy(r_loc[1][:], r_psum[1][:])
    for b in range(B):
        nc.scalar.dma_start(cc2_in[b:b + 1, :], r_loc[b][:])
    if NO_COLLECTIVE or NO_AR2:
        nc.gpsimd.dma_start(cc2_out[:], cc2_in[:])
    else:
        nc.gpsimd.collective_compute(
            "AllReduce", mybir.AluOpType.add, replica_groups=group,
            ins=[cc2_in.opt()], outs=[cc2_out.opt()],
        )
    r_sb = [pool.tile([1, D], F32, tag=f"rsb{b}", name=f"rsb{b}")
            for b in range(B)]
    for b in range(B):
        nc.scalar.dma_start(r_sb[b][:], cc2_out[b:b + 1, :])

    # broadcast r[b, :] to 128 partitions and write the output slice:
    # every row of out[b] is r[b, :].
    for b in range(B):
        r_bc = pool.tile([128, D], F32, tag=f"rb{b}")
        nc.gpsimd.partition_broadcast(r_bc[:], r_sb[b][:])
        for sc in range(2):
            c = b * 2 + sc
            nc.scalar.dma_start(out_d[c * 128:(c + 1) * 128, :], r_bc[:])


def build(loop_k: int = 0, num_devices: int = N_CORES, compile: bool = True):
    """Build + compile the SPMD program; loop_k > 1 statically unrolls the
    body that many times (timing builds)."""
    nc = bacc.Bacc("TRN2", target_bir_lowering=False, debug=False,
                   num_devices=num_devices)
    _ITER[0] = 0
    if REMOTE_AR1:
        nc._rs1 = nc.alloc_semaphore("rs1")
        nc._ls1 = nc.alloc_semaphore("ls1")
        nc._prep1 = nc.alloc_semaphore("prep1")
        nc.has_collectives = True
    h_d = nc.dram_tensor("h", [B * S_LOC, D], F32, kind="ExternalInput")
    wvt_d = nc.dram_tensor("wvt", [D, J_LOC], F32, kind="ExternalInput")
    wot_d = nc.dram_tensor("wot", [J_LOC, D], F32, kind="ExternalInput")
    out_d = nc.dram_tensor("out", [B * S_LOC, D], F32, kind="ExternalOutput")

    with tile.TileContext(nc) as tc:
        with (
            tc.tile_pool(name="sbuf", bufs=2) as pool,
            tc.tile_pool(name="psum1", bufs=2, space="PSUM") as psum1,
            tc.tile_pool(name="psum2", bufs=2, space="PSUM") as psum2,
            tc.tile_pool(name="dram", bufs=2, space="DRAM") as dram,
        ):
            pools = (pool, psum1, psum2, dram)
            const = _emit_const(nc, tc, pool, wvt_d, wot_d)
            n = max(1, loop_k)
            if n == 1 or DMA_ONLY:
                for _ in range(n):
                    _emit_body(nc, tc, pools, const, h_d, out_d)
            else:
                # software-pipelined emission (depth 2): fronts run two
                # iterations ahead of backs, so input DMA + AllReduce #1 of
                # later iterations overlap the back half of earlier ones.
                pending = _emit_front(nc, tc, pools, const, h_d, out_d)
                for _ in range(n - 1):
                    nxt = _emit_front(nc, tc, pools, const, h_d, out_d)
                    _emit_back(nc, tc, pools, pending, out_d)
                    pending = nxt
                _emit_back(nc, tc, pools, pending, out_d)
    if compile:
        nc.compile()
    return nc


def _get(loop_k: int = 0):
    if loop_k not in _BUILT:
        _BUILT[loop_k] = build(loop_k)
    return _BUILT[loop_k]


def make_in_maps(hidden_states, Wv, Wo):
    hidden_states = np.asarray(hidden_states, dtype=np.float32)
    Wv = np.asarray(Wv, dtype=np.float32)
    Wo = np.asarray(Wo, dtype=np.float32)
    in_maps = []
    for c in range(N_CORES):
        sl = slice(c * S_LOC, (c + 1) * S_LOC)
        jl = slice(c * J_LOC, (c + 1) * J_LOC)
        in_maps.append({
            "h": np.ascontiguousarray(hidden_states[:, sl, :]).reshape(B * S_LOC, D),
            "wvt": np.ascontiguousarray(Wv[jl, :].T),
            "wot": np.ascontiguousarray(Wo[:, jl].T),
        })
    return in_maps


def assemble(results):
    out = np.empty((B, S, D), np.float32)
    for c in range(N_CORES):
        o = results[c]["out"].reshape(B, S_LOC, D)
        out[:, c * S_LOC:(c + 1) * S_LOC, :] = o
    return out


def kernel(hidden_states, Wq=None, Wk=None, Wv=None, Wo=None, **_unused):
    nc = _get(0)
    in_maps = make_in_maps(hidden_states, Wv, Wo)
    res = run_bass_kernel_spmd(nc, in_maps, list(range(N_CORES)))
    return assemble(res.results)


if __name__ == "__main__":
    rng = np.random.default_rng(0)
    h = rng.standard_normal((B, S, D), dtype=np.float32)
    wv = (rng.standard_normal((D, D), dtype=np.float32) * 0.02)
    wo = (rng.standard_normal((D, D), dtype=np.float32) * 0.02)
    out = kernel(h, None, None, wv, wo)
    ref = (h.mean(axis=1) @ wv.T @ wo.T)[:, None, :] * np.ones((1, S, 1), np.float32)
    err = np.abs(out - ref).max() / np.abs(ref).max()
    print("self-check rel err:", err)



# revision 2
# speedup vs baseline: 2.2759x; 2.2759x over previous
"""Trainium2 Bass kernel for nn_Attention_50989851738305 — v3.

Math: jnp.tril(scores, k=-999999) zeroes every score, so softmax is uniform
and out[b, s, :] = mean_s'(hidden[b, s', :]) @ Wv.T @ Wo.T (constant in s).

v3 vs v2 (19.4us): queue discipline + deeper pipeline so the AllReduce no
longer head-of-line-blocks the input loads:
  - sync queue: h loads + AR-return DMA (3-deep pipeline gives it slack),
  - scalar queue: AR staging + output stores + ACT-side broadcast copies,
  - gpsimd: the AllReduce only,
  - fp16 AR payload (4KB).
"""
import numpy as np

import concourse.bass as bass  # noqa: F401
import concourse.tile as tile
from concourse import bacc, mybir
from concourse.bass_utils import run_bass_kernel_spmd

B = 2
S = 2048
D = 1024
N_CORES = 8
S_LOC = S // N_CORES          # 256 sequence rows per core
J_LOC = D // N_CORES          # 128 output columns per core
NCH = B * S_LOC // 128        # 4 input row-chunks
OCH = B * S // 128            # 32 output row-chunks
DC = D // 128                 # 8 chunks of the model dim
SCALE = 1.0 / S               # exact power of two, fp16-representable
F16 = mybir.dt.float16
F32 = mybir.dt.float32
DEPTH = 3                     # software pipeline depth (fronts ahead of backs)

_BUILT = {}
NO_COLLECTIVE = False  # timing experiment: AllReduce -> local copy
DMA_ONLY = False       # timing experiment: loads + stores only


def _emit_const(nc, tc, pool, psum_c, wv_d, wot_d, eb_d):
    """Once per NEFF: constants + folded weight slice."""
    ones_col = pool.tile([128, 1], F16, tag="ones_col", bufs=1)
    nc.vector.memset(ones_col[:], SCALE)
    # eb[p, b*128 + i] = 1 if p == b else 0 — per-b lhsT that broadcasts
    # row b of rhs to all 128 output partitions (host-provided constant).
    e_all = pool.tile([B, B * 128], F16, tag="eball", bufs=1)
    nc.sync.dma_start(e_all[:], eb_d[:])
    eb = [e_all[:, b * 128:(b + 1) * 128] for b in range(B)]

    wv_sb = pool.tile([128, DC, D], F16, tag="wv", bufs=1)
    nc.sync.dma_start(wv_sb[:], wv_d[:])
    wot_sb = pool.tile([128, DC, J_LOC], F16, tag="wot", bufs=1)
    nc.sync.dma_start(wot_sb[:], wot_d[:])

    # WfoldT[d, j] = sum_i Wv[i, d] * Wo[jslice[j], i]  — fp32 psum, fp16 SBUF
    wfold_sb = pool.tile([128, DC, J_LOC], F16, tag="wfold", bufs=1)
    for dc in range(DC):
        pw = psum_c.tile([128, J_LOC], F32, tag="pw")
        for ic in range(DC):
            nc.tensor.matmul(
                pw[:],
                wv_sb[:, ic, dc * 128:(dc + 1) * 128],
                wot_sb[:, ic, :],
                start=(ic == 0),
                stop=(ic == DC - 1),
            )
        nc.vector.tensor_copy(wfold_sb[:, dc, :], pw[:])
    return ones_col, eb, wfold_sb


def _emit_front(nc, tc, pools, const, h_d, out_d):
    """Input DMA + local partial sums + AllReduce dispatch. Returns cc_out
    handle (fp16 mean arrives there) or None in DMA_ONLY mode."""
    pool, psum1, psum2, dram = pools
    ones_col, eb, wfold_sb = const
    group = [list(range(N_CORES))]

    h_sb = pool.tile([128, NCH, D], F16, tag="hbig")
    nc.sync.dma_start(h_sb[:], h_d[:])

    if DMA_ONLY:
        nc.scalar.dma_start(out_d[:, 0:NCH * 8, :], h_sb[:].rearrange(
            "p c (k j) -> p (c k) j", j=J_LOC))
        return None

    # fold the two 128-row chunks of each batch (rows 0-255 are b=0)
    hsum = []
    for b in range(B):
        t = pool.tile([128, D], F16, tag=f"hs{b}")
        nc.vector.tensor_tensor(t[:], h_sb[:, 2 * b, :], h_sb[:, 2 * b + 1, :],
                                mybir.AluOpType.add)
        hsum.append(t)

    # pT[p, dc*2 + b] = (1/S) * sum_{s local} h[b, s, dc*128 + p]
    pT_psum = psum1.tile([128, B * DC], F32, tag="pT")
    for b in range(B):
        for dc in range(DC):
            col = dc * 2 + b
            nc.tensor.matmul(
                pT_psum[:, col:col + 1],
                hsum[b][:, dc * 128:(dc + 1) * 128],
                ones_col[:],
                start=True,
                stop=True,
            )

    # AllReduce: full-sequence mean (transposed layout), 4KB fp16
    cc_in = dram.tile([128, B * DC], F16, tag="cci")
    cc_out = dram.tile([128, B * DC], F16, tag="cco", addr_space="Shared")
    pT_loc = pool.tile([128, B * DC], F16, tag="pTl")
    nc.vector.tensor_copy(pT_loc[:], pT_psum[:])
    nc.scalar.dma_start(cc_in[:], pT_loc[:])
    if NO_COLLECTIVE:
        nc.gpsimd.dma_start(cc_out[:], cc_in[:])
    else:
        nc.gpsimd.collective_compute(
            "AllReduce", mybir.AluOpType.add, replica_groups=group,
            ins=[cc_in.opt()], outs=[cc_out.opt()],
        )
    return cc_out


def _emit_back(nc, tc, pools, cc_out, const, out_d):
    """AR return, weight matmul, row broadcast, output DMA."""
    pool, psum1, psum2, dram = pools
    ones_col, eb, wfold_sb = const

    pT16 = pool.tile([128, B * DC], F16, tag="pT16")
    nc.sync.dma_start(pT16[:], cc_out[:])

    # r[b, j] = sum_d m[b, d] * WfoldT[d, j]   (local j slice)
    r_psum = psum1.tile([B, J_LOC], F32, tag="rp")
    for dc in range(DC):
        nc.tensor.matmul(
            r_psum[:],
            pT16[:, dc * 2:dc * 2 + 2],
            wfold_sb[:, dc, :],
            start=(dc == 0),
            stop=(dc == DC - 1),
        )
    r_sb = pool.tile([B, J_LOC], F16, tag="rs")
    nc.vector.tensor_copy(r_sb[:], r_psum[:])

    # broadcast r[b, :] to all 128 partitions via PE (e_b @ r)
    bc_psum = psum2.tile([128, B, J_LOC], F32, tag="bc")
    for b in range(B):
        nc.tensor.matmul(bc_psum[:, b, :], eb[b][:], r_sb[:],
                         start=True, stop=True)

    # materialize [128, 32, 128] fp16 (chunk c holds rows of b = c // 16):
    # initial psum->sbuf copy, then doubling. b=0 on DVE, b=1 on ACT.
    rbig = pool.tile([128, OCH, J_LOC], F16, tag="rbig")
    eng = [nc.vector, nc.scalar]
    for b in range(B):
        o = b * (OCH // B)
        e = eng[b]
        if b == 0:
            e.tensor_copy(rbig[:, o, :], bc_psum[:, b, :])
        else:
            e.copy(rbig[:, o, :], bc_psum[:, b, :])
        w = 1
        while w < OCH // B:
            if b == 0:
                e.tensor_copy(rbig[:, o + w:o + 2 * w, :], rbig[:, o:o + w, :])
            else:
                e.copy(rbig[:, o + w:o + 2 * w, :], rbig[:, o:o + w, :])
            w *= 2

    nc.scalar.dma_start(out_d[:], rbig[:])


def _emit_body(nc, tc, pools, const, h_d, out_d):
    cc_out = _emit_front(nc, tc, pools, const, h_d, out_d)
    if cc_out is not None:
        _emit_back(nc, tc, pools, cc_out, const, out_d)


def build(loop_k: int = 0, num_devices: int = N_CORES, compile: bool = True):
    nc = bacc.Bacc("TRN2", target_bir_lowering=False, debug=False,
                   num_devices=num_devices)
    h_d = nc.dram_tensor("h", [128, NCH, D], F16, kind="ExternalInput")
    wv_d = nc.dram_tensor("wv", [128, DC, D], F16, kind="ExternalInput")
    wot_d = nc.dram_tensor("wot", [128, DC, J_LOC], F16, kind="ExternalInput")
    eb_d = nc.dram_tensor("eb", [B, B * 128], F16, kind="ExternalInput")
    out_d = nc.dram_tensor("out", [128, OCH, J_LOC], F16, kind="ExternalOutput")

    with tile.TileContext(nc) as tc:
        with (
            tc.tile_pool(name="sbuf", bufs=DEPTH) as pool,
            tc.tile_pool(name="psum_c", bufs=1, space="PSUM") as psum_c,
            tc.tile_pool(name="psum1", bufs=2, space="PSUM") as psum1,
            tc.tile_pool(name="psum2", bufs=2, space="PSUM") as psum2,
            tc.tile_pool(name="dram", bufs=DEPTH, space="DRAM") as dram,
        ):
            pools = (pool, psum1, psum2, dram)
            const = _emit_const(nc, tc, pool, psum_c, wv_d, wot_d, eb_d)
            n = max(1, loop_k)
            if n <= DEPTH - 1 or DMA_ONLY:
                for _ in range(n):
                    _emit_body(nc, tc, pools, const, h_d, out_d)
            else:
                pend = []
                for _ in range(DEPTH - 1):
                    pend.append(_emit_front(nc, tc, pools, const, h_d, out_d))
                for _ in range(n - DEPTH + 1):
                    nxt = _emit_front(nc, tc, pools, const, h_d, out_d)
                    _emit_back(nc, tc, pools, pend.pop(0), const, out_d)
                    pend.append(nxt)
                while pend:
                    _emit_back(nc, tc, pools, pend.pop(0), const, out_d)
    if compile:
        nc.compile()
    return nc


def _get(loop_k: int = 0):
    if loop_k not in _BUILT:
        _BUILT[loop_k] = build(loop_k)
    return _BUILT[loop_k]


def make_in_maps(hidden_states, Wv, Wo):
    h16 = np.asarray(hidden_states, dtype=np.float16)
    wv16 = np.asarray(Wv, dtype=np.float16)
    wo16 = np.asarray(Wo, dtype=np.float16)
    wv_t = np.ascontiguousarray(
        wv16.reshape(DC, 128, D).transpose(1, 0, 2))          # [128, DC, D]
    eb = np.zeros((B, B * 128), np.float16)
    for b in range(B):
        eb[b, b * 128:(b + 1) * 128] = 1.0
    in_maps = []
    for c in range(N_CORES):
        sl = slice(c * S_LOC, (c + 1) * S_LOC)
        jl = slice(c * J_LOC, (c + 1) * J_LOC)
        harr = np.ascontiguousarray(
            h16[:, sl, :].reshape(NCH, 128, D).transpose(1, 0, 2))
        wot = np.ascontiguousarray(
            wo16[jl, :].T.reshape(DC, 128, J_LOC).transpose(1, 0, 2))
        in_maps.append({"h": harr, "wv": wv_t, "wot": wot, "eb": eb})
    return in_maps


def assemble(results):
    out16 = np.empty((B, S, D), np.float16)
    for c in range(N_CORES):
        jl = slice(c * J_LOC, (c + 1) * J_LOC)
        a = results[c]["out"]                                  # [128, OCH, J]
        rows = a.transpose(1, 0, 2).reshape(B, S, J_LOC)
        out16[:, :, jl] = rows
    return out16.astype(np.float32)


def kernel(hidden_states, Wq=None, Wk=None, Wv=None, Wo=None, **_unused):
    nc = _get(0)
    in_maps = make_in_maps(hidden_states, Wv, Wo)
    res = run_bass_kernel_spmd(nc, in_maps, list(range(N_CORES)))
    return assemble(res.results)


if __name__ == "__main__":
    rng = np.random.default_rng(0)
    h = rng.standard_normal((B, S, D), dtype=np.float32)
    wv = (rng.standard_normal((D, D), dtype=np.float32) * 0.02)
    wo = (rng.standard_normal((D, D), dtype=np.float32) * 0.02)
    out = kernel(h, None, None, wv, wo)
    ref = (h.mean(axis=1) @ wv.T @ wo.T)[:, None, :] * np.ones((1, S, 1), np.float32)
    err = np.abs(out - ref).max() / np.abs(ref).max()
    print("self-check rel err:", err)
